# revision 25
# baseline (speedup 1.0000x reference)
"""Trainium2 Bass kernel for nn_CNN4CH (3x stride-2 conv -> GAP -> MLP -> 3x3 Procrustes).

Strategy (pure data parallelism, 4 samples per core on 8 cores):
  - Host: pad x, build conv1 im2col layout x100[(c,dy,dx), yo, g] covering 2x2
    output-pixel blocks (K=100, stride-4 windows), cast to fp8e4 (weights bf16).
  - Device per sample:
      conv1: K=100 matmul per row-pair into 4-bank PSUM quads; one fused
             bias+relu evacuation per quad (FD=1216) into h1ph[128, 89, 305].
      conv2: weight-outer shift-matmuls over 4-pair PSUM chunks; col-tiled
             row pairs interleaved so both 64-wide PE halves co-stream; one
             evacuation per chunk into h2ph[128, 45, 306].
      conv3: weight-outer over 4-group PSUM chunks (A: K=128 taps dy1/dy2,
             B: K=64 tap dy0); fused ReLU+bias+row-sum (accum_out) per chunk.
      GAP -> hbar[128, SPC] fp32 -> DRAM.
  - Host: FC1/FC2 + SVD -> closest-rotation projection (exact ref math).
"""

import numpy as np
import ml_dtypes
from contextlib import ExitStack

BF16 = ml_dtypes.bfloat16
F8E4 = ml_dtypes.float8_e4m3

B, CIN, H, W = 32, 4, 352, 1216
NCORES = 8
SPC = B // NCORES            # samples per core
H1, W1 = 176, 608            # conv1 out
H2, W2 = 88, 304             # conv2 out
H3, W3 = 44, 152             # conv3 out
YO, G = H1 // 2, W1 // 2     # conv1 row-pair / col-pair grid = 88 x 304
K1 = 100                     # c(4) * dy(5) * dx(5)
CHUNK = 22                   # conv1 yo rows per DMA chunk (88 = 4*22)
Q1 = 2                       # conv1 yo rows per PSUM tile / evac
C2G = 2                      # conv2 row-pairs per PSUM chunk
G3 = 3                       # conv3 output rows per matmul group
NG3 = (H3 + G3 - 1) // G3    # 15 groups (14x3 + 1x2)
C3G = 2                      # conv3 groups per PSUM chunk
POOLN = H3 * W3              # 6688 spatial positions averaged

_CACHE = {}


def _build_device(repeat=1):
    if ("nc", repeat) in _CACHE:
        return _CACHE[("nc", repeat)]
    import concourse.bass as bass
    import concourse.bacc as bacc
    import concourse.tile as tile
    import concourse.mybir as mybir

    dt = mybir.dt
    AF = mybir.ActivationFunctionType
    ALU = mybir.AluOpType

    nc = bacc.Bacc(
        "TRN2", target_bir_lowering=False, debug=False,
        enable_asserts=False, num_devices=NCORES,
    )

    # ---- DRAM I/O ----
    x100_d = nc.dram_tensor("x100", [SPC, K1, YO, W2], dt.float8e4, kind="ExternalInput")
    w1l_d = nc.dram_tensor("w1l", [K1, 128], dt.bfloat16, kind="ExternalInput")
    w2l_d = nc.dram_tensor("w2l", [128, 4 * 64], dt.bfloat16, kind="ExternalInput")
    w3a_d = nc.dram_tensor("w3a", [128, 3 * 128], dt.bfloat16, kind="ExternalInput")
    w3b_d = nc.dram_tensor("w3b", [64, 3 * 128], dt.bfloat16, kind="ExternalInput")
    b1_d = nc.dram_tensor("b1", [128, 1], dt.float32, kind="ExternalInput")
    b2_d = nc.dram_tensor("b2", [128, 1], dt.float32, kind="ExternalInput")
    b3_d = nc.dram_tensor("b3", [128, 1], dt.float32, kind="ExternalInput")
    hbar_d = nc.dram_tensor("hbar_out", [128, SPC], dt.float32, kind="ExternalOutput")

    # ---- persistent SBUF ----
    h1ph_t = nc.alloc_sbuf_tensor("h1ph", [128, YO + 1, W2 + 1], dt.bfloat16)
    h2ph_t = nc.alloc_sbuf_tensor("h2ph", [128, H3 + 1, W2 + 2], dt.bfloat16)
    w1l_t = nc.alloc_sbuf_tensor("w1l_s", [K1, 128], dt.bfloat16)
    w2l_t = nc.alloc_sbuf_tensor("w2l_s", [128, 4 * 64], dt.bfloat16)
    w3a_t = nc.alloc_sbuf_tensor("w3a_s", [128, 3 * 128], dt.bfloat16)
    w3b_t = nc.alloc_sbuf_tensor("w3b_s", [128, 3 * 128], dt.bfloat16)
    b1_t = nc.alloc_sbuf_tensor("b1_s", [128, 1], dt.float32)
    b2_t = nc.alloc_sbuf_tensor("b2_s", [128, 1], dt.float32)
    b3_t = nc.alloc_sbuf_tensor("b3_s", [128, 1], dt.float32)
    h3sums_t = nc.alloc_sbuf_tensor("h3sums", [128, 16], dt.float32)
    hbar_t = nc.alloc_sbuf_tensor("hbar", [128, SPC], dt.float32)

    h1ph = h1ph_t.ap()
    h2ph = h2ph_t.ap()

    SHIFTS = [(0, 0), (0, -1), (-1, 0), (-1, -1)]

    with TileCtx(tile, nc) as (ctx, tc):
        # weight/bias loads
        nc.sync.dma_start(w1l_t.ap()[:], w1l_d.ap()[:])
        nc.sync.dma_start(w2l_t.ap()[:], w2l_d.ap()[:])
        nc.sync.dma_start(w3a_t.ap()[:], w3a_d.ap()[:])
        nc.sync.dma_start(w3b_t.ap()[64:128, :], w3b_d.ap()[:])
        nc.sync.dma_start(b1_t.ap()[:], b1_d.ap()[:])
        nc.sync.dma_start(b2_t.ap()[:], b2_d.ap()[:])
        nc.sync.dma_start(b3_t.ap()[:], b3_d.ap()[:])
        # zero halos (only borders are ever read as padding)
        nc.gpsimd.memset(h1ph[:, 0, :], 0.0)
        nc.gpsimd.memset(h1ph[:, :, 0:1], 0.0)
        nc.gpsimd.memset(h2ph[:, 0, :], 0.0)
        nc.gpsimd.memset(h2ph[:, :, 0:1], 0.0)
        nc.gpsimd.memset(h3sums_t.ap()[:], 0.0)

        xpool = ctx.enter_context(tc.tile_pool(name="xch", bufs=4))
        pp = ctx.enter_context(tc.tile_pool(name="pp", bufs=2, space="PSUM"))
        pp3 = ctx.enter_context(tc.tile_pool(name="pp3", bufs=2, space="PSUM"))
        scr = ctx.enter_context(tc.tile_pool(name="h3scr", bufs=2))

        def bias_relu(engine, out, in0, bias):
            if engine == "act":
                nc.scalar.activation(out, in0, AF.Relu, bias=bias)
            else:
                nc.vector.tensor_scalar(out=out, in0=in0, scalar1=bias,
                                        scalar2=0.0, op0=ALU.add, op1=ALU.max)

        x100 = x100_d.ap()
        NQ1 = YO // Q1                # 44 conv1 tiles
        NC2 = H2 // 2 // C2G          # 22 conv2 chunks
        nch3 = (NG3 + C3G - 1) // C3G  # 8 conv3 chunks

        def conv1_tile(s, xts, q, pool=None):
            pl = pool or pp
            ps = pl.tile([128, Q1, 512], dt.float32,
                         tag="pp" if pl is pp else "pp3")
            for g in range(Q1):
                yo = q * Q1 + g
                xt = xts[yo // CHUNK]
                nc.tensor.matmul(ps[:, g, 0:W2], w1l_t.ap()[:],
                                 xt[:, yo % CHUNK, :], start=True, stop=True)
            bias_relu("act" if q % 2 == 0 else "dve",
                      h1ph[:, 1 + q * Q1:1 + (q + 1) * Q1, 1:W2 + 1],
                      ps[:, :, 0:W2], b1_t.ap()[:])

        def conv2_chunk(c2):
            ps = pp.tile([128, C2G, 512], dt.float32, tag="pp")
            for k, (sy, sx) in enumerate(SHIFTS):
                wk = w2l_t.ap()[:, k * 64:(k + 1) * 64]
                for g in range(C2G):
                    Yo = 2 * (c2 * C2G + g)
                    nc.tensor.matmul(
                        ps[0:64, g, 0:W2], wk,
                        h1ph[:, 1 + Yo + sy, 1 + sx:1 + sx + W2],
                        start=(k == 0), stop=False, tile_position=(0, 0),
                        skip_group_check=True,
                    )
                    nc.tensor.matmul(
                        ps[64:128, g, 0:W2], wk,
                        h1ph[:, 2 + Yo + sy, 1 + sx:1 + sx + W2],
                        start=(k == 0), stop=(k == 3), tile_position=(0, 64),
                        skip_group_check=True,
                    )
            bias_relu("act" if c2 % 2 == 0 else "dve",
                      h2ph[:, 1 + c2 * C2G:1 + (c2 + 1) * C2G, 1:W2 + 1],
                      ps[:, :, 0:W2], b2_t.ap()[:])

        def conv3_chunk(c3, slot, filler=None):
            """4 groups (last chunk 3: groups 12-14) over two 2-bank tiles.
            `filler()` is called between weight blocks to emit interleaved
            conv1 work for the next sample (keeps the PE fed during evacs)."""
            g0 = c3 * 4
            gs = min(4, NG3 - g0)
            ps3a = pp3.tile([128, 2, 512], dt.float32, tag="pp3", name="ps3a")
            ps3b = pp3.tile([128, 2, 512], dt.float32, tag="pp3", name="ps3b")
            pss = [ps3a, ps3b]
            rows_n = [((g0 + g) * G3, min(G3, H3 - (g0 + g) * G3))
                      for g in range(gs)]
            for dxp in range(3):
                if filler:
                    filler()
                wa = w3a_t.ap()[:, dxp * 128:(dxp + 1) * 128]
                for g in range(gs):
                    y0, rows = rows_n[g]
                    nc.tensor.matmul(
                        pss[g // 2][:, g % 2, 0:rows * W3], wa,
                        h2ph[:, 1 + y0:1 + y0 + rows, dxp:dxp + 2 * W3:2],
                        start=(dxp == 0), stop=False,
                        skip_group_check=True,
                    )
            for dxp in range(3):
                if filler:
                    filler()
                wb = w3b_t.ap()[64:128, dxp * 128:(dxp + 1) * 128]
                for g in range(gs):
                    y0, rows = rows_n[g]
                    nc.tensor.matmul(
                        pss[g // 2][:, g % 2, 0:rows * W3], wb,
                        h2ph[64:128, y0:y0 + rows, dxp:dxp + 2 * W3:2],
                        start=False, stop=(dxp == 2),
                        skip_group_check=True,
                    )
            # bias+relu+sum per tile; tiles alternate ACT (fused accum) and
            # DVE (tensor_scalar + reduce) to split the evacuation load.
            for t in range((gs + 1) // 2):
                tg = rows_n[2 * t:2 * t + 2]
                nfull = sum(1 for _, r in tg if r == G3)
                h3s = scr.tile([128, 2, G3 * W3], dt.bfloat16, tag="h3scr")
                spans = []
                if nfull > 0:
                    spans.append((slice(0, nfull), G3 * W3, True))
                if nfull < len(tg):
                    spans.append((slice(nfull, nfull + 1), tg[nfull][1] * W3, False))
                for gsl, n3, full in spans:
                    src = pss[t][:, gsl, 0:n3]
                    dst = h3s[:, gsl, 0:n3] if full else h3s[:, gsl.start, 0:n3]
                    if (c3 + t) % 2 == 0:
                        nc.scalar.activation(
                            dst, src, AF.Relu, bias=b3_t.ap()[:],
                            accum_out=h3sums_t.ap()[:, slot[0]:slot[0] + 1])
                    else:
                        nc.vector.tensor_scalar(
                            out=dst, in0=src, scalar1=b3_t.ap()[:],
                            scalar2=0.0, op0=ALU.add, op1=ALU.max)
                        red_in = (dst.rearrange("p a b -> p (a b)")
                                  if full else dst)
                        nc.vector.tensor_reduce(
                            h3sums_t.ap()[:, slot[0]:slot[0] + 1], red_in,
                            axis=mybir.AxisListType.X, op=ALU.add)
                    slot[0] += 1

        nch3 = (NG3 + 3) // 4  # 4 conv3 chunks of up to 4 groups
        samples = [si for _ in range(repeat) for si in range(SPC)]

        def issue_x_dma(s):
            xts = []
            for ch in range(YO // CHUNK):
                xt = xpool.tile([K1, CHUNK, W2], dt.float8e4, tag="xch")
                nc.sync.dma_start(xt[:], x100[s, :, ch * CHUNK:(ch + 1) * CHUNK, :])
                xts.append(xt)
            return xts

        def conv3_sample(s, slot, filler=None):
            for c3 in range(nch3):
                conv3_chunk(c3, slot, filler=filler)
            nc.vector.tensor_reduce(hbar_t.ap()[:, s:s + 1],
                                    h3sums_t.ap()[:, 0:16],
                                    axis=mybir.AxisListType.X, op=ALU.add)

        # steady state: conv3 of sample i-1 runs interleaved with conv1 of
        # sample i (conv3 is PE-heavy/evac-light, conv1 the opposite), then
        # conv2 of sample i; x-chunks for sample i+1 prefetch under conv2.
        xts = issue_x_dma(samples[0])
        for i, s in enumerate(samples):
            slot = [0]
            if i == 0:
                # no prior conv3 to hide behind: use both PSUM pools for a
                # 4-deep conv1 pipeline instead
                for q in range(NQ1):
                    conv1_tile(s, xts, q, pool=(pp if q % 2 == 0 else pp3))
            else:
                pend = list(range(NQ1))

                def filler():
                    for q in pend[:2]:
                        conv1_tile(s, xts, q)
                    del pend[:2]

                conv3_sample(samples[i - 1], prev_slot, filler=filler)
                for q in pend:
                    conv1_tile(s, xts, q)
            if i + 1 < len(samples):
                xts = issue_x_dma(samples[i + 1])
            for j in range(NC2):
                conv2_chunk(j)
            prev_slot = slot
        conv3_sample(samples[-1], prev_slot)
        nc.sync.dma_start(hbar_d.ap()[:], hbar_t.ap()[:])

    nc.compile()
    _CACHE[("nc", repeat)] = nc
    return nc


class TileCtx:
    """ExitStack + TileContext combined context manager."""

    def __init__(self, tile_mod, nc):
        self.tile_mod = tile_mod
        self.nc = nc

    def __enter__(self):
        self.ctx = ExitStack()
        self.tc = self.tile_mod.TileContext(self.nc)
        self.tc.__enter__()
        return self.ctx, self.tc

    def __exit__(self, *exc):
        try:
            self.ctx.close()
        finally:
            return self.tc.__exit__(*exc)


def _host_prepare(x, wc1, bc1, wc2, bc2, wc3, bc3, wl1, bl1, wl2, bl2):
    """Build per-core input maps (im2col'd x + weight layouts)."""
    xp = np.pad(np.asarray(x, dtype=np.float32), ((0, 0), (0, 0), (1, 1), (1, 1)))
    sN, sC, sH, sW = xp.strides
    # x100[b, c, dy, dx, yo, g] = xp[b, c, 4*yo+dy, 4*g+dx]
    win = np.lib.stride_tricks.as_strided(
        xp, (B, CIN, 5, 5, YO, G), (sN, sC, sH, sW, 4 * sH, 4 * sW))
    x100 = np.ascontiguousarray(win.reshape(B, K1, YO, G)).astype(F8E4)

    # conv1 weights: lhsT [100, 128]; m = r*64 + j*32 + co; p = c*25 + dy*5 + dx
    w1l = np.zeros((K1, 128), np.float32)
    for r in range(2):
        for j in range(2):
            for dyp in range(3):
                for dxp in range(3):
                    dy, dx = 2 * r + dyp, 2 * j + dxp
                    for c in range(CIN):
                        p = c * 25 + dy * 5 + dx
                        w1l[p, r * 64 + j * 32 + np.arange(32)] = wc1[:, c, dyp, dxp]

    # conv2 shift weights: [128, 4*64]; partition p = yph*64 + xph*32 + c
    SHIFTS = [(0, 0), (0, -1), (-1, 0), (-1, -1)]
    w2l = np.zeros((128, 4 * 64), np.float32)
    for k, (sy, sx) in enumerate(SHIFTS):
        for yph in range(2):
            for xph in range(2):
                if sy == 0:
                    dyp = 1 if yph == 0 else 2
                elif yph == 1:
                    dyp = 0
                else:
                    continue
                if sx == 0:
                    dxp = 1 if xph == 0 else 2
                elif xph == 1:
                    dxp = 0
                else:
                    continue
                for c in range(32):
                    w2l[yph * 64 + xph * 32 + c, k * 64:(k + 1) * 64] = wc2[:, c, dyp, dxp]

    # conv3: A [128, 3*128] (yph0 -> dy'=1, yph1 -> dy'=2); B [64, 3*128] (dy'=0)
    w3a = np.zeros((128, 3 * 128), np.float32)
    w3b = np.zeros((64, 3 * 128), np.float32)
    for dxp in range(3):
        for c in range(64):
            w3a[c, dxp * 128:(dxp + 1) * 128] = wc3[:, c, 1, dxp]
            w3a[64 + c, dxp * 128:(dxp + 1) * 128] = wc3[:, c, 2, dxp]
            w3b[c, dxp * 128:(dxp + 1) * 128] = wc3[:, c, 0, dxp]

    b1 = np.tile(np.asarray(bc1, np.float32), 4).reshape(128, 1)
    b2 = np.tile(np.asarray(bc2, np.float32), 2).reshape(128, 1)
    b3 = np.asarray(bc3, np.float32).reshape(128, 1)

    shared = {
        "w1l": w1l.astype(BF16), "w2l": w2l.astype(BF16),
        "w3a": w3a.astype(BF16), "w3b": w3b.astype(BF16),
        "b1": b1, "b2": b2, "b3": b3,
    }
    in_maps = []
    for core in range(NCORES):
        m = dict(shared)
        m["x100"] = np.ascontiguousarray(x100[core * SPC:(core + 1) * SPC])
        in_maps.append(m)
    return in_maps


def _procrustes(r):
    R = r.reshape(-1, 3, 3).astype(np.float32)
    U, _, Vh = np.linalg.svd(R)
    det = np.linalg.det(U @ Vh)
    U[:, :, -1] *= np.sign(det)[:, None]
    return (U @ Vh).astype(np.float32)


def _host_tail(hbar, wl1, bl1, wl2, bl2):
    """hbar: [B, 128] pooled sums (not yet divided by POOLN)."""
    h = hbar.astype(np.float32) / float(POOLN)
    h = np.maximum(h @ np.asarray(wl1, np.float32).T + np.asarray(bl1, np.float32), 0)
    r = h @ np.asarray(wl2, np.float32).T + np.asarray(bl2, np.float32)
    return _procrustes(r)


def kernel(**inputs):
    from concourse.bass_utils import run_bass_kernel_spmd
    nc = _build_device()
    in_maps = _host_prepare(**inputs)
    res = run_bass_kernel_spmd(nc, in_maps, list(range(NCORES)))
    hbar = np.concatenate(
        [res.results[i]["hbar_out"].T for i in range(NCORES)], axis=0)
    return _host_tail(hbar, inputs["wl1"], inputs["bl1"], inputs["wl2"], inputs["bl2"])


if __name__ == "__main__":
    d = np.load("inputs.npz")
    out = kernel(**{k: d[k] for k in d.files})
    exp = np.load("expected.npy")
    err = np.abs(out - exp).max()
    print("absmax err:", err, "rel:", err / np.abs(exp).max())


# revision 26
# speedup vs baseline: 1.0183x; 1.0183x over previous
"""Trainium2 Bass kernel for nn_CNN4CH (3x stride-2 conv -> GAP -> MLP -> 3x3 Procrustes).

Strategy (pure data parallelism, 4 samples per core on 8 cores):
  - Host: pad x, build conv1 im2col layout x100[(c,dy,dx), yo, g] covering 2x2
    output-pixel blocks (K=100, stride-4 windows), cast to fp8e4 (weights bf16).
  - Device per sample:
      conv1: K=100 matmul per row-pair into 4-bank PSUM quads; one fused
             bias+relu evacuation per quad (FD=1216) into h1ph[128, 89, 305].
      conv2: weight-outer shift-matmuls over 4-pair PSUM chunks; col-tiled
             row pairs interleaved so both 64-wide PE halves co-stream; one
             evacuation per chunk into h2ph[128, 45, 306].
      conv3: weight-outer over 4-group PSUM chunks (A: K=128 taps dy1/dy2,
             B: K=64 tap dy0); fused ReLU+bias+row-sum (accum_out) per chunk.
      GAP -> hbar[128, SPC] fp32 -> DRAM.
  - Host: FC1/FC2 + SVD -> closest-rotation projection (exact ref math).
"""

import numpy as np
import ml_dtypes
from contextlib import ExitStack

BF16 = ml_dtypes.bfloat16
F8E4 = ml_dtypes.float8_e4m3

B, CIN, H, W = 32, 4, 352, 1216
NCORES = 8
SPC = B // NCORES            # samples per core
H1, W1 = 176, 608            # conv1 out
H2, W2 = 88, 304             # conv2 out
H3, W3 = 44, 152             # conv3 out
YO, G = H1 // 2, W1 // 2     # conv1 row-pair / col-pair grid = 88 x 304
K1 = 100                     # c(4) * dy(5) * dx(5)
CHUNK = 22                   # conv1 yo rows per DMA chunk (88 = 4*22)
Q1 = 2                       # conv1 yo rows per PSUM tile / evac
C2G = 2                      # conv2 row-pairs per PSUM chunk
G3 = 3                       # conv3 output rows per matmul group
NG3 = (H3 + G3 - 1) // G3    # 15 groups (14x3 + 1x2)
C3G = 2                      # conv3 groups per PSUM chunk
POOLN = H3 * W3              # 6688 spatial positions averaged

_CACHE = {}


def _build_device(repeat=1):
    if ("nc", repeat) in _CACHE:
        return _CACHE[("nc", repeat)]
    import concourse.bass as bass
    import concourse.bacc as bacc
    import concourse.tile as tile
    import concourse.mybir as mybir

    dt = mybir.dt
    AF = mybir.ActivationFunctionType
    ALU = mybir.AluOpType

    nc = bacc.Bacc(
        "TRN2", target_bir_lowering=False, debug=False,
        enable_asserts=False, num_devices=NCORES,
    )

    # ---- DRAM I/O ----
    x100_d = nc.dram_tensor("x100", [SPC, K1, YO, W2], dt.float8e4, kind="ExternalInput")
    w1l_d = nc.dram_tensor("w1l", [K1, 128], dt.bfloat16, kind="ExternalInput")
    w2l_d = nc.dram_tensor("w2l", [128, 4 * 64], dt.bfloat16, kind="ExternalInput")
    w3a_d = nc.dram_tensor("w3a", [128, 3 * 128], dt.bfloat16, kind="ExternalInput")
    w3b_d = nc.dram_tensor("w3b", [64, 3 * 128], dt.bfloat16, kind="ExternalInput")
    b1_d = nc.dram_tensor("b1", [128, 1], dt.float32, kind="ExternalInput")
    b2_d = nc.dram_tensor("b2", [128, 1], dt.float32, kind="ExternalInput")
    b3_d = nc.dram_tensor("b3", [128, 1], dt.float32, kind="ExternalInput")
    hbar_d = nc.dram_tensor("hbar_out", [128, SPC], dt.float32, kind="ExternalOutput")

    # ---- persistent SBUF ----
    h1ph_t = nc.alloc_sbuf_tensor("h1ph", [128, YO + 1, W2 + 1], dt.bfloat16)
    h2ph_t = nc.alloc_sbuf_tensor("h2ph", [128, H3 + 1, W2 + 2], dt.bfloat16)
    w1l_t = nc.alloc_sbuf_tensor("w1l_s", [K1, 128], dt.bfloat16)
    w2l_t = nc.alloc_sbuf_tensor("w2l_s", [128, 4 * 64], dt.bfloat16)
    w3a_t = nc.alloc_sbuf_tensor("w3a_s", [128, 3 * 128], dt.bfloat16)
    w3b_t = nc.alloc_sbuf_tensor("w3b_s", [128, 3 * 128], dt.bfloat16)
    b1_t = nc.alloc_sbuf_tensor("b1_s", [128, 1], dt.float32)
    b2_t = nc.alloc_sbuf_tensor("b2_s", [128, 1], dt.float32)
    b3_t = nc.alloc_sbuf_tensor("b3_s", [128, 1], dt.float32)
    h3sums_t = nc.alloc_sbuf_tensor("h3sums", [128, 16], dt.float32)
    hbar_t = nc.alloc_sbuf_tensor("hbar", [128, SPC], dt.float32)

    h1ph = h1ph_t.ap()
    h2ph = h2ph_t.ap()

    SHIFTS = [(0, 0), (0, -1), (-1, 0), (-1, -1)]

    with TileCtx(tile, nc) as (ctx, tc):
        # weight/bias loads
        nc.sync.dma_start(w1l_t.ap()[:], w1l_d.ap()[:])
        nc.sync.dma_start(w2l_t.ap()[:], w2l_d.ap()[:])
        nc.sync.dma_start(w3a_t.ap()[:], w3a_d.ap()[:])
        nc.sync.dma_start(w3b_t.ap()[64:128, :], w3b_d.ap()[:])
        nc.sync.dma_start(b1_t.ap()[:], b1_d.ap()[:])
        nc.sync.dma_start(b2_t.ap()[:], b2_d.ap()[:])
        nc.sync.dma_start(b3_t.ap()[:], b3_d.ap()[:])
        # zero halos (only borders are ever read as padding)
        nc.gpsimd.memset(h1ph[:, 0, :], 0.0)
        nc.gpsimd.memset(h1ph[:, :, 0:1], 0.0)
        nc.gpsimd.memset(h2ph[:, 0, :], 0.0)
        nc.gpsimd.memset(h2ph[:, :, 0:1], 0.0)
        nc.gpsimd.memset(h3sums_t.ap()[:], 0.0)

        xpool = ctx.enter_context(tc.tile_pool(name="xch", bufs=4))
        pp = ctx.enter_context(tc.tile_pool(name="pp", bufs=2, space="PSUM"))
        pp3 = ctx.enter_context(tc.tile_pool(name="pp3", bufs=2, space="PSUM"))
        scr = ctx.enter_context(tc.tile_pool(name="h3scr", bufs=2))

        def bias_relu(engine, out, in0, bias):
            if engine == "act":
                nc.scalar.activation(out, in0, AF.Relu, bias=bias)
            else:
                nc.vector.tensor_scalar(out=out, in0=in0, scalar1=bias,
                                        scalar2=0.0, op0=ALU.add, op1=ALU.max)

        x100 = x100_d.ap()
        NQ1 = YO // Q1                # 44 conv1 tiles
        NC2 = H2 // 2 // C2G          # 22 conv2 chunks
        nch3 = (NG3 + C3G - 1) // C3G  # 8 conv3 chunks

        def conv1_tile(s, xts, q, pool=None):
            pl = pool or pp
            ps = pl.tile([128, Q1, 512], dt.float32,
                         tag="pp" if pl is pp else "pp3")
            for g in range(Q1):
                yo = q * Q1 + g
                xt = xts[yo // CHUNK]
                nc.tensor.matmul(ps[:, g, 0:W2], w1l_t.ap()[:],
                                 xt[:, yo % CHUNK, :], start=True, stop=True)
            bias_relu("act" if q % 2 == 0 else "dve",
                      h1ph[:, 1 + q * Q1:1 + (q + 1) * Q1, 1:W2 + 1],
                      ps[:, :, 0:W2], b1_t.ap()[:])

        def conv2_chunk(c2):
            pl = pp if c2 % 2 == 0 else pp3
            ps = pl.tile([128, C2G, 512], dt.float32,
                         tag="pp" if pl is pp else "pp3")
            for k, (sy, sx) in enumerate(SHIFTS):
                wk = w2l_t.ap()[:, k * 64:(k + 1) * 64]
                for g in range(C2G):
                    Yo = 2 * (c2 * C2G + g)
                    nc.tensor.matmul(
                        ps[0:64, g, 0:W2], wk,
                        h1ph[:, 1 + Yo + sy, 1 + sx:1 + sx + W2],
                        start=(k == 0), stop=False, tile_position=(0, 0),
                        skip_group_check=True,
                    )
                    nc.tensor.matmul(
                        ps[64:128, g, 0:W2], wk,
                        h1ph[:, 2 + Yo + sy, 1 + sx:1 + sx + W2],
                        start=(k == 0), stop=(k == 3), tile_position=(0, 64),
                        skip_group_check=True,
                    )
            bias_relu("act" if c2 % 2 == 0 else "dve",
                      h2ph[:, 1 + c2 * C2G:1 + (c2 + 1) * C2G, 1:W2 + 1],
                      ps[:, :, 0:W2], b2_t.ap()[:])

        def conv3_chunk(c3, slot, filler=None):
            """4 groups (last chunk 3: groups 12-14) over two 2-bank tiles.
            `filler()` is called between weight blocks to emit interleaved
            conv1 work for the next sample (keeps the PE fed during evacs)."""
            g0 = c3 * 4
            gs = min(4, NG3 - g0)
            ps3a = pp3.tile([128, 2, 512], dt.float32, tag="pp3", name="ps3a")
            ps3b = pp3.tile([128, 2, 512], dt.float32, tag="pp3", name="ps3b")
            pss = [ps3a, ps3b]
            rows_n = [((g0 + g) * G3, min(G3, H3 - (g0 + g) * G3))
                      for g in range(gs)]
            for dxp in range(3):
                if filler:
                    filler()
                wa = w3a_t.ap()[:, dxp * 128:(dxp + 1) * 128]
                for g in range(gs):
                    y0, rows = rows_n[g]
                    nc.tensor.matmul(
                        pss[g // 2][:, g % 2, 0:rows * W3], wa,
                        h2ph[:, 1 + y0:1 + y0 + rows, dxp:dxp + 2 * W3:2],
                        start=(dxp == 0), stop=False,
                        skip_group_check=True,
                    )
            for dxp in range(3):
                if filler:
                    filler()
                wb = w3b_t.ap()[64:128, dxp * 128:(dxp + 1) * 128]
                for g in range(gs):
                    y0, rows = rows_n[g]
                    nc.tensor.matmul(
                        pss[g // 2][:, g % 2, 0:rows * W3], wb,
                        h2ph[64:128, y0:y0 + rows, dxp:dxp + 2 * W3:2],
                        start=False, stop=(dxp == 2),
                        skip_group_check=True,
                    )
            # bias+relu+sum per tile; tiles alternate ACT (fused accum) and
            # DVE (tensor_scalar + reduce) to split the evacuation load.
            for t in range((gs + 1) // 2):
                tg = rows_n[2 * t:2 * t + 2]
                nfull = sum(1 for _, r in tg if r == G3)
                h3s = scr.tile([128, 2, G3 * W3], dt.bfloat16, tag="h3scr")
                spans = []
                if nfull > 0:
                    spans.append((slice(0, nfull), G3 * W3, True))
                if nfull < len(tg):
                    spans.append((slice(nfull, nfull + 1), tg[nfull][1] * W3, False))
                for gsl, n3, full in spans:
                    src = pss[t][:, gsl, 0:n3]
                    dst = h3s[:, gsl, 0:n3] if full else h3s[:, gsl.start, 0:n3]
                    if (c3 + t) % 2 == 0:
                        nc.scalar.activation(
                            dst, src, AF.Relu, bias=b3_t.ap()[:],
                            accum_out=h3sums_t.ap()[:, slot[0]:slot[0] + 1])
                    else:
                        nc.vector.tensor_scalar(
                            out=dst, in0=src, scalar1=b3_t.ap()[:],
                            scalar2=0.0, op0=ALU.add, op1=ALU.max)
                        red_in = (dst.rearrange("p a b -> p (a b)")
                                  if full else dst)
                        nc.vector.tensor_reduce(
                            h3sums_t.ap()[:, slot[0]:slot[0] + 1], red_in,
                            axis=mybir.AxisListType.X, op=ALU.add)
                    slot[0] += 1

        nch3 = (NG3 + 3) // 4  # 4 conv3 chunks of up to 4 groups
        samples = [si for _ in range(repeat) for si in range(SPC)]

        def issue_x_dma(s):
            xts = []
            for ch in range(YO // CHUNK):
                xt = xpool.tile([K1, CHUNK, W2], dt.float8e4, tag="xch")
                nc.sync.dma_start(xt[:], x100[s, :, ch * CHUNK:(ch + 1) * CHUNK, :])
                xts.append(xt)
            return xts

        def conv3_sample(s, slot, filler=None):
            for c3 in range(nch3):
                conv3_chunk(c3, slot, filler=filler)
            nc.vector.tensor_reduce(hbar_t.ap()[:, s:s + 1],
                                    h3sums_t.ap()[:, 0:16],
                                    axis=mybir.AxisListType.X, op=ALU.add)

        # steady state: conv3 of sample i-1 runs interleaved with conv1 of
        # sample i (conv3 is PE-heavy/evac-light, conv1 the opposite), then
        # conv2 of sample i; x-chunks for sample i+1 prefetch under conv2.
        xts = issue_x_dma(samples[0])
        for i, s in enumerate(samples):
            slot = [0]
            if i == 0:
                # no prior conv3 to hide behind: use both PSUM pools for a
                # 4-deep conv1 pipeline instead
                for q in range(NQ1):
                    conv1_tile(s, xts, q, pool=(pp if q % 2 == 0 else pp3))
            else:
                pend = list(range(NQ1))

                def filler():
                    for q in pend[:2]:
                        conv1_tile(s, xts, q)
                    del pend[:2]

                conv3_sample(samples[i - 1], prev_slot, filler=filler)
                for q in pend:
                    conv1_tile(s, xts, q)
            if i + 1 < len(samples):
                xts = issue_x_dma(samples[i + 1])
            for j in range(NC2):
                conv2_chunk(j)
            prev_slot = slot
        conv3_sample(samples[-1], prev_slot)
        nc.sync.dma_start(hbar_d.ap()[:], hbar_t.ap()[:])

    nc.compile()
    _CACHE[("nc", repeat)] = nc
    return nc


class TileCtx:
    """ExitStack + TileContext combined context manager."""

    def __init__(self, tile_mod, nc):
        self.tile_mod = tile_mod
        self.nc = nc

    def __enter__(self):
        self.ctx = ExitStack()
        self.tc = self.tile_mod.TileContext(self.nc)
        self.tc.__enter__()
        return self.ctx, self.tc

    def __exit__(self, *exc):
        try:
            self.ctx.close()
        finally:
            return self.tc.__exit__(*exc)


def _host_prepare(x, wc1, bc1, wc2, bc2, wc3, bc3, wl1, bl1, wl2, bl2):
    """Build per-core input maps (im2col'd x + weight layouts)."""
    xp = np.pad(np.asarray(x, dtype=np.float32), ((0, 0), (0, 0), (1, 1), (1, 1)))
    sN, sC, sH, sW = xp.strides
    # x100[b, c, dy, dx, yo, g] = xp[b, c, 4*yo+dy, 4*g+dx]
    win = np.lib.stride_tricks.as_strided(
        xp, (B, CIN, 5, 5, YO, G), (sN, sC, sH, sW, 4 * sH, 4 * sW))
    x100 = np.ascontiguousarray(win.reshape(B, K1, YO, G)).astype(F8E4)

    # conv1 weights: lhsT [100, 128]; m = r*64 + j*32 + co; p = c*25 + dy*5 + dx
    w1l = np.zeros((K1, 128), np.float32)
    for r in range(2):
        for j in range(2):
            for dyp in range(3):
                for dxp in range(3):
                    dy, dx = 2 * r + dyp, 2 * j + dxp
                    for c in range(CIN):
                        p = c * 25 + dy * 5 + dx
                        w1l[p, r * 64 + j * 32 + np.arange(32)] = wc1[:, c, dyp, dxp]

    # conv2 shift weights: [128, 4*64]; partition p = yph*64 + xph*32 + c
    SHIFTS = [(0, 0), (0, -1), (-1, 0), (-1, -1)]
    w2l = np.zeros((128, 4 * 64), np.float32)
    for k, (sy, sx) in enumerate(SHIFTS):
        for yph in range(2):
            for xph in range(2):
                if sy == 0:
                    dyp = 1 if yph == 0 else 2
                elif yph == 1:
                    dyp = 0
                else:
                    continue
                if sx == 0:
                    dxp = 1 if xph == 0 else 2
                elif xph == 1:
                    dxp = 0
                else:
                    continue
                for c in range(32):
                    w2l[yph * 64 + xph * 32 + c, k * 64:(k + 1) * 64] = wc2[:, c, dyp, dxp]

    # conv3: A [128, 3*128] (yph0 -> dy'=1, yph1 -> dy'=2); B [64, 3*128] (dy'=0)
    w3a = np.zeros((128, 3 * 128), np.float32)
    w3b = np.zeros((64, 3 * 128), np.float32)
    for dxp in range(3):
        for c in range(64):
            w3a[c, dxp * 128:(dxp + 1) * 128] = wc3[:, c, 1, dxp]
            w3a[64 + c, dxp * 128:(dxp + 1) * 128] = wc3[:, c, 2, dxp]
            w3b[c, dxp * 128:(dxp + 1) * 128] = wc3[:, c, 0, dxp]

    b1 = np.tile(np.asarray(bc1, np.float32), 4).reshape(128, 1)
    b2 = np.tile(np.asarray(bc2, np.float32), 2).reshape(128, 1)
    b3 = np.asarray(bc3, np.float32).reshape(128, 1)

    shared = {
        "w1l": w1l.astype(BF16), "w2l": w2l.astype(BF16),
        "w3a": w3a.astype(BF16), "w3b": w3b.astype(BF16),
        "b1": b1, "b2": b2, "b3": b3,
    }
    in_maps = []
    for core in range(NCORES):
        m = dict(shared)
        m["x100"] = np.ascontiguousarray(x100[core * SPC:(core + 1) * SPC])
        in_maps.append(m)
    return in_maps


def _procrustes(r):
    R = r.reshape(-1, 3, 3).astype(np.float32)
    U, _, Vh = np.linalg.svd(R)
    det = np.linalg.det(U @ Vh)
    U[:, :, -1] *= np.sign(det)[:, None]
    return (U @ Vh).astype(np.float32)


def _host_tail(hbar, wl1, bl1, wl2, bl2):
    """hbar: [B, 128] pooled sums (not yet divided by POOLN)."""
    h = hbar.astype(np.float32) / float(POOLN)
    h = np.maximum(h @ np.asarray(wl1, np.float32).T + np.asarray(bl1, np.float32), 0)
    r = h @ np.asarray(wl2, np.float32).T + np.asarray(bl2, np.float32)
    return _procrustes(r)


def kernel(**inputs):
    from concourse.bass_utils import run_bass_kernel_spmd
    nc = _build_device()
    in_maps = _host_prepare(**inputs)
    res = run_bass_kernel_spmd(nc, in_maps, list(range(NCORES)))
    hbar = np.concatenate(
        [res.results[i]["hbar_out"].T for i in range(NCORES)], axis=0)
    return _host_tail(hbar, inputs["wl1"], inputs["bl1"], inputs["wl2"], inputs["bl2"])


if __name__ == "__main__":
    d = np.load("inputs.npz")
    out = kernel(**{k: d[k] for k in d.files})
    exp = np.load("expected.npy")
    err = np.abs(out - exp).max()
    print("absmax err:", err, "rel:", err / np.abs(exp).max())


# revision 27
# speedup vs baseline: 1.0207x; 1.0024x over previous
"""Trainium2 Bass kernel for nn_CNN4CH (3x stride-2 conv -> GAP -> MLP -> 3x3 Procrustes).

Strategy (pure data parallelism, 4 samples per core on 8 cores):
  - Host: pad x, build conv1 im2col layout x100[(c,dy,dx), yo, g] covering 2x2
    output-pixel blocks (K=100, stride-4 windows), cast to fp8e4 (weights bf16).
  - Device per sample:
      conv1: K=100 matmul per row-pair into 4-bank PSUM quads; one fused
             bias+relu evacuation per quad (FD=1216) into h1ph[128, 89, 305].
      conv2: weight-outer shift-matmuls over 4-pair PSUM chunks; col-tiled
             row pairs interleaved so both 64-wide PE halves co-stream; one
             evacuation per chunk into h2ph[128, 45, 306].
      conv3: weight-outer over 4-group PSUM chunks (A: K=128 taps dy1/dy2,
             B: K=64 tap dy0); fused ReLU+bias+row-sum (accum_out) per chunk.
      GAP -> hbar[128, SPC] fp32 -> DRAM.
  - Host: FC1/FC2 + SVD -> closest-rotation projection (exact ref math).
"""

import numpy as np
import ml_dtypes
from contextlib import ExitStack

BF16 = ml_dtypes.bfloat16
F8E4 = ml_dtypes.float8_e4m3

B, CIN, H, W = 32, 4, 352, 1216
NCORES = 8
SPC = B // NCORES            # samples per core
H1, W1 = 176, 608            # conv1 out
H2, W2 = 88, 304             # conv2 out
H3, W3 = 44, 152             # conv3 out
YO, G = H1 // 2, W1 // 2     # conv1 row-pair / col-pair grid = 88 x 304
K1 = 100                     # c(4) * dy(5) * dx(5)
CHUNK = 22                   # conv1 yo rows per DMA chunk (88 = 4*22)
Q1 = 2                       # conv1 yo rows per PSUM tile / evac
C2G = 2                      # conv2 row-pairs per PSUM chunk
G3 = 3                       # conv3 output rows per matmul group
NG3 = (H3 + G3 - 1) // G3    # 15 groups (14x3 + 1x2)
C3G = 2                      # conv3 groups per PSUM chunk
POOLN = H3 * W3              # 6688 spatial positions averaged

_CACHE = {}


def _build_device(repeat=1):
    if ("nc", repeat) in _CACHE:
        return _CACHE[("nc", repeat)]
    import concourse.bass as bass
    import concourse.bacc as bacc
    import concourse.tile as tile
    import concourse.mybir as mybir

    dt = mybir.dt
    AF = mybir.ActivationFunctionType
    ALU = mybir.AluOpType

    nc = bacc.Bacc(
        "TRN2", target_bir_lowering=False, debug=False,
        enable_asserts=False, num_devices=NCORES,
    )

    # ---- DRAM I/O ----
    x100_d = nc.dram_tensor("x100", [SPC, K1, YO, W2], dt.float8e4, kind="ExternalInput")
    w1l_d = nc.dram_tensor("w1l", [K1, 128], dt.bfloat16, kind="ExternalInput")
    w2l_d = nc.dram_tensor("w2l", [128, 4 * 64], dt.bfloat16, kind="ExternalInput")
    w3a_d = nc.dram_tensor("w3a", [128, 3 * 128], dt.bfloat16, kind="ExternalInput")
    w3b_d = nc.dram_tensor("w3b", [64, 3 * 128], dt.bfloat16, kind="ExternalInput")
    b1_d = nc.dram_tensor("b1", [128, 1], dt.float32, kind="ExternalInput")
    b2_d = nc.dram_tensor("b2", [128, 1], dt.float32, kind="ExternalInput")
    b3_d = nc.dram_tensor("b3", [128, 1], dt.float32, kind="ExternalInput")
    hbar_d = nc.dram_tensor("hbar_out", [128, SPC], dt.float32, kind="ExternalOutput")

    # ---- persistent SBUF ----
    h1ph_t = nc.alloc_sbuf_tensor("h1ph", [128, YO + 1, W2 + 1], dt.bfloat16)
    h2ph_t = nc.alloc_sbuf_tensor("h2ph", [128, H3 + 1, W2 + 2], dt.bfloat16)
    w1l_t = nc.alloc_sbuf_tensor("w1l_s", [K1, 128], dt.bfloat16)
    w2l_t = nc.alloc_sbuf_tensor("w2l_s", [128, 4 * 64], dt.bfloat16)
    w3a_t = nc.alloc_sbuf_tensor("w3a_s", [128, 3 * 128], dt.bfloat16)
    w3b_t = nc.alloc_sbuf_tensor("w3b_s", [128, 3 * 128], dt.bfloat16)
    b1_t = nc.alloc_sbuf_tensor("b1_s", [128, 1], dt.float32)
    b2_t = nc.alloc_sbuf_tensor("b2_s", [128, 1], dt.float32)
    b3_t = nc.alloc_sbuf_tensor("b3_s", [128, 1], dt.float32)
    h3sums_t = nc.alloc_sbuf_tensor("h3sums", [128, 16], dt.float32)
    hbar_t = nc.alloc_sbuf_tensor("hbar", [128, SPC], dt.float32)

    h1ph = h1ph_t.ap()
    h2ph = h2ph_t.ap()

    SHIFTS = [(0, 0), (0, -1), (-1, 0), (-1, -1)]

    with TileCtx(tile, nc) as (ctx, tc):
        # weight/bias loads
        nc.sync.dma_start(w1l_t.ap()[:], w1l_d.ap()[:])
        nc.sync.dma_start(w2l_t.ap()[:], w2l_d.ap()[:])
        nc.sync.dma_start(w3a_t.ap()[:], w3a_d.ap()[:])
        nc.sync.dma_start(w3b_t.ap()[64:128, :], w3b_d.ap()[:])
        nc.sync.dma_start(b1_t.ap()[:], b1_d.ap()[:])
        nc.sync.dma_start(b2_t.ap()[:], b2_d.ap()[:])
        nc.sync.dma_start(b3_t.ap()[:], b3_d.ap()[:])
        # zero halos (only borders are ever read as padding)
        nc.gpsimd.memset(h1ph[:, 0, :], 0.0)
        nc.gpsimd.memset(h1ph[:, :, 0:1], 0.0)
        nc.gpsimd.memset(h2ph[:, 0, :], 0.0)
        nc.gpsimd.memset(h2ph[:, :, 0:1], 0.0)
        nc.gpsimd.memset(h3sums_t.ap()[:], 0.0)

        xpool = ctx.enter_context(tc.tile_pool(name="xch", bufs=4))
        pp = ctx.enter_context(tc.tile_pool(name="pp", bufs=2, space="PSUM"))
        pp3 = ctx.enter_context(tc.tile_pool(name="pp3", bufs=2, space="PSUM"))
        scr = ctx.enter_context(tc.tile_pool(name="h3scr", bufs=2))

        def bias_relu(engine, out, in0, bias):
            if engine == "act":
                nc.scalar.activation(out, in0, AF.Relu, bias=bias)
            else:
                nc.vector.tensor_scalar(out=out, in0=in0, scalar1=bias,
                                        scalar2=0.0, op0=ALU.add, op1=ALU.max)

        x100 = x100_d.ap()
        NQ1 = YO // Q1                # 44 conv1 tiles
        NC2 = H2 // 2 // C2G          # 22 conv2 chunks
        nch3 = (NG3 + C3G - 1) // C3G  # 8 conv3 chunks

        def conv1_tile(s, xts, q, pool=None):
            pl = pool or pp
            ps = pl.tile([128, Q1, 512], dt.float32,
                         tag="pp" if pl is pp else "pp3")
            for g in range(Q1):
                yo = q * Q1 + g
                xt = xts[yo // CHUNK]
                nc.tensor.matmul(ps[:, g, 0:W2], w1l_t.ap()[:],
                                 xt[:, yo % CHUNK, :], start=True, stop=True)
            bias_relu("act" if q % 2 == 0 else "dve",
                      h1ph[:, 1 + q * Q1:1 + (q + 1) * Q1, 1:W2 + 1],
                      ps[:, :, 0:W2], b1_t.ap()[:])

        def conv2_chunk(c2):
            pl = pp if c2 % 2 == 0 else pp3
            ps = pl.tile([128, C2G, 512], dt.float32,
                         tag="pp" if pl is pp else "pp3")
            for k, (sy, sx) in enumerate(SHIFTS):
                wk = w2l_t.ap()[:, k * 64:(k + 1) * 64]
                for g in range(C2G):
                    Yo = 2 * (c2 * C2G + g)
                    nc.tensor.matmul(
                        ps[0:64, g, 0:W2], wk,
                        h1ph[:, 1 + Yo + sy, 1 + sx:1 + sx + W2],
                        start=(k == 0), stop=False, tile_position=(0, 0),
                        skip_group_check=True,
                    )
                    nc.tensor.matmul(
                        ps[64:128, g, 0:W2], wk,
                        h1ph[:, 2 + Yo + sy, 1 + sx:1 + sx + W2],
                        start=(k == 0), stop=(k == 3), tile_position=(0, 64),
                        skip_group_check=True,
                    )
            bias_relu("act" if c2 % 2 == 0 else "dve",
                      h2ph[:, 1 + c2 * C2G:1 + (c2 + 1) * C2G, 1:W2 + 1],
                      ps[:, :, 0:W2], b2_t.ap()[:])

        def conv3_chunk(c3, slot, filler=None):
            """4 groups (last chunk 3: groups 12-14) over two 2-bank tiles.
            `filler()` is called between weight blocks to emit interleaved
            conv1 work for the next sample (keeps the PE fed during evacs)."""
            g0 = c3 * 4
            gs = min(4, NG3 - g0)
            ps3a = pp3.tile([128, 2, 512], dt.float32, tag="pp3", name="ps3a")
            ps3b = pp3.tile([128, 2, 512], dt.float32, tag="pp3", name="ps3b")
            pss = [ps3a, ps3b]
            rows_n = [((g0 + g) * G3, min(G3, H3 - (g0 + g) * G3))
                      for g in range(gs)]
            for dxp in range(3):
                if filler:
                    filler()
                wa = w3a_t.ap()[:, dxp * 128:(dxp + 1) * 128]
                for g in range(gs):
                    y0, rows = rows_n[g]
                    nc.tensor.matmul(
                        pss[g // 2][:, g % 2, 0:rows * W3], wa,
                        h2ph[:, 1 + y0:1 + y0 + rows, dxp:dxp + 2 * W3:2],
                        start=(dxp == 0), stop=False,
                        skip_group_check=True,
                    )
            for dxp in range(3):
                if filler:
                    filler()
                wb = w3b_t.ap()[64:128, dxp * 128:(dxp + 1) * 128]
                for g in range(gs):
                    y0, rows = rows_n[g]
                    nc.tensor.matmul(
                        pss[g // 2][:, g % 2, 0:rows * W3], wb,
                        h2ph[64:128, y0:y0 + rows, dxp:dxp + 2 * W3:2],
                        start=False, stop=(dxp == 2),
                        skip_group_check=True,
                    )
            # bias+relu+sum per tile; tiles alternate ACT (fused accum) and
            # DVE (tensor_scalar + reduce) to split the evacuation load.
            for t in range((gs + 1) // 2):
                tg = rows_n[2 * t:2 * t + 2]
                nfull = sum(1 for _, r in tg if r == G3)
                h3s = scr.tile([128, 2, G3 * W3], dt.bfloat16, tag="h3scr")
                spans = []
                if nfull > 0:
                    spans.append((slice(0, nfull), G3 * W3, True))
                if nfull < len(tg):
                    spans.append((slice(nfull, nfull + 1), tg[nfull][1] * W3, False))
                for gsl, n3, full in spans:
                    src = pss[t][:, gsl, 0:n3]
                    dst = h3s[:, gsl, 0:n3] if full else h3s[:, gsl.start, 0:n3]
                    if (c3 + t) % 2 == 0:
                        nc.scalar.activation(
                            dst, src, AF.Relu, bias=b3_t.ap()[:],
                            accum_out=h3sums_t.ap()[:, slot[0]:slot[0] + 1])
                    else:
                        nc.vector.tensor_scalar(
                            out=dst, in0=src, scalar1=b3_t.ap()[:],
                            scalar2=0.0, op0=ALU.add, op1=ALU.max)
                        red_in = (dst.rearrange("p a b -> p (a b)")
                                  if full else dst)
                        nc.vector.tensor_reduce(
                            h3sums_t.ap()[:, slot[0]:slot[0] + 1], red_in,
                            axis=mybir.AxisListType.X, op=ALU.add)
                    slot[0] += 1

        nch3 = (NG3 + 3) // 4  # 4 conv3 chunks of up to 4 groups
        samples = [si for _ in range(repeat) for si in range(SPC)]

        def issue_x_dma(s):
            xts = []
            for ch in range(YO // CHUNK):
                xt = xpool.tile([K1, CHUNK, W2], dt.float8e4, tag="xch")
                nc.sync.dma_start(xt[:], x100[s, :, ch * CHUNK:(ch + 1) * CHUNK, :])
                xts.append(xt)
            return xts

        def conv3_sample(s, slot, filler=None):
            for c3 in range(nch3):
                conv3_chunk(c3, slot, filler=filler)
            nc.vector.tensor_reduce(hbar_t.ap()[:, s:s + 1],
                                    h3sums_t.ap()[:, 0:16],
                                    axis=mybir.AxisListType.X, op=ALU.add)

        # steady state: conv3 of sample i-1 runs interleaved with conv1 of
        # sample i (conv3 is PE-heavy/evac-light, conv1 the opposite), then
        # conv2 of sample i; x-chunks for sample i+1 prefetch under conv2.
        xts = issue_x_dma(samples[0])
        for i, s in enumerate(samples):
            slot = [0]
            if i == 0:
                # no prior conv3 to hide behind: use both PSUM pools for a
                # 4-deep conv1 pipeline instead
                for q in range(NQ1):
                    conv1_tile(s, xts, q, pool=(pp if q % 2 == 0 else pp3))
            else:
                pend = list(range(NQ1))

                def filler():
                    for q in pend[:2]:
                        conv1_tile(s, xts, q)
                    del pend[:2]

                conv3_sample(samples[i - 1], prev_slot, filler=filler)
                for q in pend:
                    conv1_tile(s, xts, q)
            for j in range(NC2):
                if j == 6 and i + 1 < len(samples):
                    xts = issue_x_dma(samples[i + 1])
                conv2_chunk(j)
            prev_slot = slot
        conv3_sample(samples[-1], prev_slot)
        nc.sync.dma_start(hbar_d.ap()[:], hbar_t.ap()[:])

    nc.compile()
    _CACHE[("nc", repeat)] = nc
    return nc


class TileCtx:
    """ExitStack + TileContext combined context manager."""

    def __init__(self, tile_mod, nc):
        self.tile_mod = tile_mod
        self.nc = nc

    def __enter__(self):
        self.ctx = ExitStack()
        self.tc = self.tile_mod.TileContext(self.nc)
        self.tc.__enter__()
        return self.ctx, self.tc

    def __exit__(self, *exc):
        try:
            self.ctx.close()
        finally:
            return self.tc.__exit__(*exc)


def _host_prepare(x, wc1, bc1, wc2, bc2, wc3, bc3, wl1, bl1, wl2, bl2):
    """Build per-core input maps (im2col'd x + weight layouts)."""
    xp = np.pad(np.asarray(x, dtype=np.float32), ((0, 0), (0, 0), (1, 1), (1, 1)))
    sN, sC, sH, sW = xp.strides
    # x100[b, c, dy, dx, yo, g] = xp[b, c, 4*yo+dy, 4*g+dx]
    win = np.lib.stride_tricks.as_strided(
        xp, (B, CIN, 5, 5, YO, G), (sN, sC, sH, sW, 4 * sH, 4 * sW))
    x100 = np.ascontiguousarray(win.reshape(B, K1, YO, G)).astype(F8E4)

    # conv1 weights: lhsT [100, 128]; m = r*64 + j*32 + co; p = c*25 + dy*5 + dx
    w1l = np.zeros((K1, 128), np.float32)
    for r in range(2):
        for j in range(2):
            for dyp in range(3):
                for dxp in range(3):
                    dy, dx = 2 * r + dyp, 2 * j + dxp
                    for c in range(CIN):
                        p = c * 25 + dy * 5 + dx
                        w1l[p, r * 64 + j * 32 + np.arange(32)] = wc1[:, c, dyp, dxp]

    # conv2 shift weights: [128, 4*64]; partition p = yph*64 + xph*32 + c
    SHIFTS = [(0, 0), (0, -1), (-1, 0), (-1, -1)]
    w2l = np.zeros((128, 4 * 64), np.float32)
    for k, (sy, sx) in enumerate(SHIFTS):
        for yph in range(2):
            for xph in range(2):
                if sy == 0:
                    dyp = 1 if yph == 0 else 2
                elif yph == 1:
                    dyp = 0
                else:
                    continue
                if sx == 0:
                    dxp = 1 if xph == 0 else 2
                elif xph == 1:
                    dxp = 0
                else:
                    continue
                for c in range(32):
                    w2l[yph * 64 + xph * 32 + c, k * 64:(k + 1) * 64] = wc2[:, c, dyp, dxp]

    # conv3: A [128, 3*128] (yph0 -> dy'=1, yph1 -> dy'=2); B [64, 3*128] (dy'=0)
    w3a = np.zeros((128, 3 * 128), np.float32)
    w3b = np.zeros((64, 3 * 128), np.float32)
    for dxp in range(3):
        for c in range(64):
            w3a[c, dxp * 128:(dxp + 1) * 128] = wc3[:, c, 1, dxp]
            w3a[64 + c, dxp * 128:(dxp + 1) * 128] = wc3[:, c, 2, dxp]
            w3b[c, dxp * 128:(dxp + 1) * 128] = wc3[:, c, 0, dxp]

    b1 = np.tile(np.asarray(bc1, np.float32), 4).reshape(128, 1)
    b2 = np.tile(np.asarray(bc2, np.float32), 2).reshape(128, 1)
    b3 = np.asarray(bc3, np.float32).reshape(128, 1)

    shared = {
        "w1l": w1l.astype(BF16), "w2l": w2l.astype(BF16),
        "w3a": w3a.astype(BF16), "w3b": w3b.astype(BF16),
        "b1": b1, "b2": b2, "b3": b3,
    }
    in_maps = []
    for core in range(NCORES):
        m = dict(shared)
        m["x100"] = np.ascontiguousarray(x100[core * SPC:(core + 1) * SPC])
        in_maps.append(m)
    return in_maps


def _procrustes(r):
    R = r.reshape(-1, 3, 3).astype(np.float32)
    U, _, Vh = np.linalg.svd(R)
    det = np.linalg.det(U @ Vh)
    U[:, :, -1] *= np.sign(det)[:, None]
    return (U @ Vh).astype(np.float32)


def _host_tail(hbar, wl1, bl1, wl2, bl2):
    """hbar: [B, 128] pooled sums (not yet divided by POOLN)."""
    h = hbar.astype(np.float32) / float(POOLN)
    h = np.maximum(h @ np.asarray(wl1, np.float32).T + np.asarray(bl1, np.float32), 0)
    r = h @ np.asarray(wl2, np.float32).T + np.asarray(bl2, np.float32)
    return _procrustes(r)


def kernel(**inputs):
    from concourse.bass_utils import run_bass_kernel_spmd
    nc = _build_device()
    in_maps = _host_prepare(**inputs)
    res = run_bass_kernel_spmd(nc, in_maps, list(range(NCORES)))
    hbar = np.concatenate(
        [res.results[i]["hbar_out"].T for i in range(NCORES)], axis=0)
    return _host_tail(hbar, inputs["wl1"], inputs["bl1"], inputs["wl2"], inputs["bl2"])


if __name__ == "__main__":
    d = np.load("inputs.npz")
    out = kernel(**{k: d[k] for k in d.files})
    exp = np.load("expected.npy")
    err = np.abs(out - exp).max()
    print("absmax err:", err, "rel:", err / np.abs(exp).max())
